# revision 4
# baseline (speedup 1.0000x reference)
"""CPC unsupervised criterion loss on 8 Trainium2 NeuronCores.

Strategy (data-parallel over batch B=8, one batch row per core):
  - The irregular 121 MB negative-sample gather is replaced by a dense
    score matrix: for each (k, w) we compute scores against ALL B*S=1024
    encoder rows via PE matmuls in bf16 (4x PE rate). Sampled-negative
    multiplicities cnt[w,j] are built on the host from the index tensors.
  - Per k, the 116x1024 fp32 score matrix in PSUM is post-processed with
    exactly three full passes, one per engine class:
      * ScalarE: Exp PSUM -> SBUF bf16 (no mask needed: exp > 0, and the
        mask/cnt multiplies below zero out unsampled columns).
      * VectorE tensor_tensor_reduce (16-bit 2x mode): msk01 * exp with
        max-accumulate -> maxexp[w] = max over sampled of exp(score).
      * VectorE scalar_tensor_tensor (16-bit 2x mode): cnt * exp with
        sum-accumulate -> negsum[w] = sum_n exp(negScore_n) exactly
        (duplicates weighted by cnt).
    The old design's 4 fp16 identity matmuls per k (bias/ln-cnt adds into
    PSUM) and the 1.2us PSUM reduce_max are gone.
  - The positive score is extracted bit-exactly from the PSUM score row
    via a one-hot multiply + sum (scalar_tensor_tensor with accum_out).
  - Accuracy bits are decided in exp domain on host: acc = exp(pos) >=
    maxexp. bf16-exp noise (2^-8 rel) is far below the host re-check
    window: near-margin positions (|pos - ln maxexp| < 0.02) are
    re-resolved exactly in float64 on the host, including the exact-tie
    case (positive drawn as its own negative).
  - cT is zero-padded to 2x128 columns so locC matmuls run N=128 and the
    score matmuls' stationary operand is a full 128-column bf16 weight
    (fast-weight-load eligible). Score PSUM rows 116:127 are zeros and
    ignored by the host.
"""

import numpy as np

B, S, K, D, NNEG = 8, 128, 12, 256, 128
W = S - K          # 116
J = B * S          # 1024
NCORES = 8
MARGIN_TAU = 0.02        # host re-check window around pos ~ maxneg (ln domain)

# bf16 blob columns: flatT (ec-major) | cT (padded to 2x128)
BB_FT, BB_CT = 0, 2 * J
BB_END = BB_CT + 2 * 128                             # 2304
# bf16 mask blob columns: msk01 | cnt
MK_MSK, MK_CNT = 0, J
MK_END = 2 * J                                       # 2048

_CACHE = {}


def _build_program():
    from concourse import bacc, mybir
    import concourse.tile as tile

    f32 = mybir.dt.float32
    bf16 = mybir.dt.bfloat16
    Alu = mybir.AluOpType
    Act = mybir.ActivationFunctionType

    nc = bacc.Bacc(
        "TRN2", target_bir_lowering=False, debug=False, num_devices=NCORES
    )

    fb_d = nc.dram_tensor("fblob", [128, S + K], f32, kind="ExternalInput")
    bb_d = nc.dram_tensor("bblob", [128, BB_END], bf16, kind="ExternalInput")
    mk_d = nc.dram_tensor("mblob", [128, MK_END], bf16, kind="ExternalInput")
    wp_d = nc.dram_tensor("wpredT", [128, K * 2 * D], bf16, kind="ExternalInput")
    out_d = nc.dram_tensor("out", [128, 3 * K], f32, kind="ExternalOutput")

    with tile.TileContext(nc) as tc:
        with (
            tc.tile_pool(name="consts", bufs=1) as consts,
            tc.tile_pool(name="lcpool", bufs=3) as lcpool,
            tc.tile_pool(name="scr", bufs=2) as scr,
            tc.tile_pool(name="outs", bufs=1) as outs,
            tc.tile_pool(name="pslc", bufs=2, space="PSUM") as pslc,
            tc.tile_pool(name="pssc", bufs=3, space="PSUM") as pssc,
        ):
            bb = consts.tile([128, BB_END], bf16)
            wpall = consts.tile([128, K * 2 * D], bf16)
            fb = consts.tile([128, S + K], f32)
            mk = consts.tile([128, MK_END], bf16)
            # ordered so the first k's dependencies land first
            nc.sync.dma_start(bb[:, BB_CT:BB_END], bb_d[:, BB_CT:BB_END])
            nc.sync.dma_start(wpall[:, 0:512], wp_d[:, 0:512])
            nc.sync.dma_start(bb[:, 0:J], bb_d[:, 0:J])
            nc.sync.dma_start(fb[:], fb_d[:])
            nc.sync.dma_start(bb[:, J:2 * J], bb_d[:, J:2 * J])
            nc.sync.dma_start(mk[:, 0:J], mk_d[:, 0:J])
            nc.sync.dma_start(mk[:, J:MK_END], mk_d[:, J:MK_END])
            nc.sync.dma_start(wpall[:, 512:2048], wp_d[:, 512:2048])
            nc.sync.dma_start(wpall[:, 2048:K * 512], wp_d[:, 2048:K * 512])

            fT_v = bb[:, BB_FT:BB_FT + 2 * J]
            cT_v = bb[:, BB_CT:BB_CT + 2 * 128]
            msk_v = mk[:, MK_MSK:MK_MSK + J]
            cnt_v = mk[:, MK_CNT:MK_CNT + J]

            posS = outs.tile([128, K], f32)
            maxexp = outs.tile([128, K], f32)
            negsum = outs.tile([128, K], f32)

            for k in range(K):
                wk = wpall[:, k * 2 * D:(k + 1) * 2 * D]

                # locC_T[k]: (e', ec*128 + w) = sum_d WpredT[d, e] * cT[d, w]
                lcT_ps = pslc.tile([128, 256], f32, tag="lcT")
                for ec in range(2):
                    for dc in range(2):
                        nc.tensor.matmul(
                            lcT_ps[:, ec * 128:(ec + 1) * 128],
                            lhsT=wk[:, dc * D + ec * 128: dc * D + (ec + 1) * 128],
                            rhs=cT_v[:, dc * 128:(dc + 1) * 128],
                            start=(dc == 0),
                            stop=(dc == 1),
                        )
                lcT_bf = lcpool.tile([128, 256], bf16, tag="lcT_bf")
                if k % 2 == 0:
                    nc.scalar.copy(lcT_bf[:], lcT_ps[:])
                else:
                    nc.vector.tensor_copy(lcT_bf[:], lcT_ps[:])

                # scores (bf16): (w, j) = sum_e locC_T[e, w] * flatT[e, j]
                # ec-outer so each stationary weight is loaded once
                sc_ps = pssc.tile([128, J], f32, tag="sc")
                for ec in range(2):
                    for jc in range(2):
                        nc.tensor.matmul(
                            sc_ps[:, jc * 512:(jc + 1) * 512],
                            lhsT=lcT_bf[:, ec * 128:(ec + 1) * 128],
                            rhs=fT_v[:, ec * J + jc * 512: ec * J + (jc + 1) * 512],
                            start=(ec == 0),
                            stop=(ec == 1),
                            skip_group_check=True,
                        )

                # positive score: exact one-hot extraction at column k+1+w
                scrP = scr.tile([128, S], f32, tag="scrP")
                nc.vector.scalar_tensor_tensor(
                    out=scrP[:],
                    in0=sc_ps[:, 0:S],
                    scalar=1.0,
                    in1=fb[:, K - k:K - k + S],
                    op0=Alu.mult,
                    op1=Alu.mult,
                    accum_out=posS[:, k:k + 1],
                )

                # exp(score) -> SBUF bf16 (raw scores; masking happens below)
                scrB = scr.tile([128, J], bf16, tag="scrB")
                nc.scalar.activation(
                    out=scrB[:],
                    in_=sc_ps[:],
                    func=Act.Exp,
                )

                # maxexp = max over sampled of exp(score): 16-bit 2x mask
                # multiply, then a reduce_max (TTR's microcode is add-only —
                # op1=max is an invalid instruction and wedges the device)
                scrM = scr.tile([128, J], bf16, tag="scrM")
                nc.vector.tensor_tensor(
                    out=scrM[:],
                    in0=scrB[:],
                    in1=msk_v[:],
                    op=Alu.mult,
                )
                nc.vector.reduce_max(
                    maxexp[:, k:k + 1], scrM[:], axis=mybir.AxisListType.X
                )

                # negsum = sum_j cnt * exp(score)  (16-bit 2x pass)
                scrC = scr.tile([128, J], bf16, tag="scrC")
                nc.vector.scalar_tensor_tensor(
                    out=scrC[:],
                    in0=scrB[:],
                    scalar=1.0,
                    in1=cnt_v[:],
                    op0=Alu.mult,
                    op1=Alu.mult,
                    accum_out=negsum[:, k:k + 1],
                )

            nc.sync.dma_start(out_d[:, 0:K], negsum[:])
            nc.sync.dma_start(out_d[:, K:2 * K], posS[:])
            nc.sync.dma_start(out_d[:, 2 * K:3 * K], maxexp[:])

    nc.compile()
    return nc


def _host_prep(cFeature, encodedData, Wpred, batchIdx, seqIdx):
    import ml_dtypes

    bf = ml_dtypes.bfloat16
    cF = np.ascontiguousarray(np.asarray(cFeature, dtype=np.float32))
    eD = np.ascontiguousarray(np.asarray(encodedData, dtype=np.float32))
    Wp = np.ascontiguousarray(np.asarray(Wpred, dtype=np.float32))
    bI = np.asarray(batchIdx).astype(np.int64)
    sI = np.asarray(seqIdx).astype(np.int64)

    flat = eD.reshape(J, D)
    idx = np.arange(NNEG * W * B, dtype=np.int64)
    ext = ((sI + idx % W) % S + bI * S).reshape(B, NNEG, W)

    wt = Wp.transpose(0, 2, 1)  # (K, d, e)
    wp_host = np.concatenate(
        [np.concatenate([wt[k, :128, :], wt[k, 128:, :]], axis=1) for k in range(K)],
        axis=1,
    ).astype(bf)  # (128, K*2D)
    wp_host = np.ascontiguousarray(wp_host)

    fblob = np.zeros((128, S + K), np.float32)
    fblob[np.arange(W), np.arange(W) + K + 1] = 1.0

    rows = np.tile(np.arange(W), NNEG)
    in_maps = []
    cnts_orig = []
    for b in range(B):
        perm = np.r_[b * S:(b + 1) * S, 0:b * S, (b + 1) * S:J]
        inv = np.empty(J, np.int64)
        inv[perm] = np.arange(J)

        fT = flat[perm].T  # (D, J) fp32
        cT = cF[b, :W].T * np.float32(1.0 / 256.0)  # exact power-of-2 scale

        cnt = np.zeros((W, J), np.float32)
        np.add.at(cnt, (rows, inv[ext[b].ravel()]), 1.0)
        cnt_o = np.zeros((W, J), np.float32)
        np.add.at(cnt_o, (rows, ext[b].ravel()), 1.0)
        cnts_orig.append(cnt_o)

        bblob = np.zeros((128, BB_END), bf)
        bblob[:, BB_FT:BB_FT + J] = fT[:128].astype(bf)
        bblob[:, BB_FT + J:BB_FT + 2 * J] = fT[128:].astype(bf)
        bblob[:, BB_CT:BB_CT + W] = cT[:128].astype(bf)
        bblob[:, BB_CT + 128:BB_CT + 128 + W] = cT[128:].astype(bf)

        mblob = np.zeros((128, MK_END), bf)
        mblob[:W, MK_MSK:MK_MSK + J] = (cnt > 0).astype(bf)
        mblob[:W, MK_CNT:MK_CNT + J] = cnt.astype(bf)

        in_maps.append({
            "fblob": fblob,
            "bblob": np.ascontiguousarray(bblob),
            "mblob": np.ascontiguousarray(mblob),
            "wpredT": wp_host,
        })
    return in_maps, cnts_orig, flat, cF, Wp


def _host_fix_acc(acc01, margin, cnts_orig, flat, cF, Wp):
    """Re-resolve near-margin accuracy bits exactly in float64."""
    flat64 = flat.astype(np.float64)
    for b in range(B):
        flag = np.abs(margin[b]) < MARGIN_TAU    # (W, K)
        for w, k in zip(*np.nonzero(flag)):
            lc = (cF[b, w].astype(np.float64) / 256.0) @ Wp[k].astype(np.float64).T
            sc = flat64 @ lc                     # (J,)
            mn = sc[cnts_orig[b][w] > 0].max()
            p = sc[b * S + k + 1 + w]
            acc01[b, w, k] = 1.0 if p >= mn else 0.0
    return acc01


def kernel(cFeature, encodedData, Wpred, batchIdx, seqIdx, _trace=False):
    from concourse.bass_utils import run_bass_kernel_spmd

    in_maps, cnts_orig, flat, cF, Wp = _host_prep(
        cFeature, encodedData, Wpred, batchIdx, seqIdx
    )

    if "nc" not in _CACHE:
        _CACHE["nc"] = _build_program()
    nc = _CACHE["nc"]

    kw = {}
    if _trace:
        kw = {"trace": True}
    res = run_bass_kernel_spmd(nc, in_maps, core_ids=list(range(NCORES)), **kw)
    _CACHE["last_results"] = res

    outs = np.stack([res.results[b]["out"][:W] for b in range(B)])  # (B, W, 3K)
    negsum = outs[:, :, :K].astype(np.float64)
    posS = outs[:, :, K:2 * K]
    maxexp = outs[:, :, 2 * K:3 * K].astype(np.float64)
    p64 = posS.astype(np.float64)
    lossc = np.log(negsum + np.exp(p64)) - p64

    margin = p64 - np.log(maxexp)
    acc01 = (margin >= 0).astype(np.float32)
    acc01 = _host_fix_acc(acc01, margin, cnts_orig, flat, cF, Wp)

    losses = lossc.sum(axis=(0, 1), dtype=np.float64) / (B * W)
    accs = acc01.sum(axis=(0, 1), dtype=np.float64) / (B * W)
    return (
        losses.astype(np.float32)[None, :],
        accs.astype(np.float32)[None, :],
    )
